# revision 16
# baseline (speedup 1.0000x reference)
"""Trainium2 Bass kernel for nn_CustomLoss_87522843558003 (YOLO-style CIoU+BCE loss).

Strategy (data-parallel over batch, 8 cores), v2 anchor-streaming:
 - Each core processes 8 consecutive batches; positions map onto 128 SBUF
   partitions as [batch(8) x section(16)] rows of 525 positions.
 - DMA: 4 big ops per iteration (target + one per anchor), 31.5KB contiguous
   per partition row, alternating sync/scalar rings -> ~275 GB/s/core
   (measured; small-row chunked DMA only reaches ~120 GB/s).
 - Anchor argmax streams: per-anchor IoU vs target, forward-cascade selection
   with full-15-channel copy_predicated into the next anchor's tile
   (first-max semantics); the last tile holds the selected predictions.
 - ACT does only one batched Ln per iteration (no table thrash):
     * arctan eliminated: atan(rt)-atan(rp) = atan(u),
       u = (wt*hp-wp*ht)/(hp*ht+wt*wp); (4/pi^2)atan^2(sqrt(z)) fit by
       F(z)=z(z+b)/(z^2+cz+d), z=u^2  (end-loss err ~6e-7).
     * BCE: q = |p+t-1| for t in {0,1}; sum_c -ln q_c = -ln prod_c |d_c|
       via one abs-multiply tensor_reduce; obj BCE = -ln p_obj (t_obj=mask).
 - Post-selection CIoU recomputed once, batched 525-wide, with manual
   scratch-tile reuse (write-after-last-read).
 - Per-partition masked sums via accum_out -> [128,8] output; host normalizes.
"""

import contextlib
import numpy as np

B, A, N, CH = 64, 3, 8400, 15
NCORES = 8
BPC = B // NCORES      # batches per core
SEC = 16               # partition sections per batch
PPART = BPC * SEC      # 128 partitions
W5 = N // SEC          # 525 positions per partition row
NCLS = 10
EPS = 1e-7
# rational fit of (4/pi^2)*atan(sqrt(z))^2
FB = 18.5807497
FC = 29.74781457
FD = 47.19260109

_CACHE = {}


def _build_bass(loop_r=None, level=4, io_bufs=2, t_bufs=1, wk_bufs=1,
                d_slices=3, dma_order=("sync", "scalar", "sync", "scalar")):
    """loop_r: device-side For_i repeat count (None = single pass).
    level: -1=jumbo DMA probe, 0=DMA only, 1=+argmax, 2=+selection+bce-prep,
    3=+ciou, 4=full. dma_order: rings for (T, P0, P1, P2)."""
    import concourse.tile as tile
    import concourse.mybir as mybir
    from concourse import bacc

    Alu = mybir.AluOpType
    Act = mybir.ActivationFunctionType
    f32 = mybir.dt.float32
    i32 = mybir.dt.int32

    nc = bacc.Bacc("TRN2", target_bir_lowering=False, debug=False,
                   num_devices=NCORES)
    predL = nc.dram_tensor("predL", [BPC, A, N, CH], f32, kind="ExternalInput").ap()
    targL = nc.dram_tensor("targL", [BPC, N, CH], f32, kind="ExternalInput").ap()
    accO = nc.dram_tensor("acc_out", [PPART, 8], f32, kind="ExternalOutput").ap()

    pre = predL.rearrange("b a (s j) c -> b a s (j c)", s=SEC)
    tre = targL.rearrange("b (s j) c -> b s (j c)", s=SEC)
    FW = W5 * CH
    LD = W5 // d_slices

    with tile.TileContext(nc) as tc:
        with (
            tc.tile_pool(name="pP", bufs=io_bufs) as pP,
            tc.tile_pool(name="pT", bufs=t_bufs) as pT,
            tc.tile_pool(name="pPer", bufs=1) as pPer,
            tc.tile_pool(name="pW", bufs=wk_bufs) as pW,
            tc.tile_pool(name="pQ", bufs=1) as pQ,
            tc.tile_pool(name="pB", bufs=1) as pB,
            tc.tile_pool(name="pAcc", bufs=1) as pAcc,
        ):
            ACC = pAcc.tile([PPART, 8], f32)
            ring = {"sync": nc.sync, "scalar": nc.scalar, "gpsimd": nc.gpsimd}

            if level == -1:
                PJ = pB.tile([PPART, BPC * A * N * CH // PPART], f32)
                TJ = pB.tile([PPART, BPC * N * CH // PPART], f32)
                CN0 = pB.tile([PPART, 64], f32)
                loop_cmj = tc.For_i(0, loop_r, 1) if loop_r else contextlib.nullcontext()
                with loop_cmj:
                    nc.sync.dma_start(PJ[:], predL.rearrange("b a n c -> (b a n c)")
                                      .rearrange("(p f) -> p f", p=PPART))
                    nc.scalar.dma_start(TJ[:], targL.rearrange("b n c -> (b n c)")
                                        .rearrange("(p f) -> p f", p=PPART))
                    nc.vector.tensor_scalar(CN0[:], TJ[:, 0:64], 1.0, 0.0, Alu.mult,
                                            Alu.add, accum_out=ACC[:, 0:1])
                    nc.vector.tensor_scalar(CN0[:], PJ[:, 0:64], 1.0, 0.0, Alu.mult,
                                            Alu.add, accum_out=ACC[:, 1:2])
                nc.sync.dma_start(accO, ACC[:])

            loop_cm = (tc.For_i(0, loop_r, 1)
                       if (loop_r and level != -1) else contextlib.nullcontext())
            with loop_cm:
              if level >= 0:
                T = pT.tile([PPART, FW], f32)
                ring[dma_order[0]].dma_start(T[:], tre)
                Tr = T[:].rearrange("p (j c) -> p j c", c=CH)

                CNK = pW.tile([PPART, 64], f32) if level <= 1 else None
                if level == 0:
                    nc.vector.tensor_scalar(CNK[:], T[:, 0:64], 1.0, 0.0,
                                            Alu.mult, Alu.add,
                                            accum_out=ACC[:, 3:4])
                else:
                    # ---- target prep (Pool) + early copies to free T ----
                    TWHH = pW.tile([PPART, 2 * W5], f32)
                    TLO = pPer.tile([PPART, 2 * W5], f32)
                    THI = pPer.tile([PPART, 2 * W5], f32)
                    TA = pPer.tile([PPART, W5], f32)
                    MSK = pPer.tile([PPART, W5], f32)
                    TWH = pPer.tile([PPART, 2 * W5], f32)
                    TCT = pPer.tile([PPART, 2 * W5], f32)
                    twhhr = TWHH[:].rearrange("p (j c) -> p j c", c=2)
                    tlor = TLO[:].rearrange("p (j c) -> p j c", c=2)
                    thir = THI[:].rearrange("p (j c) -> p j c", c=2)
                    twhr = TWH[:].rearrange("p (j c) -> p j c", c=2)
                    tctr = TCT[:].rearrange("p (j c) -> p j c", c=2)
                    nc.gpsimd.tensor_scalar(twhhr, Tr[:, :, 2:4], 0.5, None, Alu.mult)
                    nc.gpsimd.tensor_tensor(tlor, Tr[:, :, 0:2], twhhr, Alu.subtract)
                    nc.gpsimd.tensor_tensor(thir, Tr[:, :, 0:2], twhhr, Alu.add)
                    nc.gpsimd.tensor_tensor(TA[:], Tr[:, :, 2], Tr[:, :, 3], Alu.mult)
                    nc.gpsimd.tensor_scalar(MSK[:], Tr[:, :, 4], 1.0, None, Alu.mult)
                    nc.gpsimd.tensor_scalar(twhr, Tr[:, :, 2:4], 1.0, None, Alu.mult)
                    nc.gpsimd.tensor_scalar(tctr, Tr[:, :, 0:2], 1.0, None, Alu.mult)

                qprev = None
                Pprev = None
                for a in range(A):
                    PT = pP.tile([PPART, FW], f32, name="PT")
                    ring[dma_order[1 + a]].dma_start(PT[:], pre[:, a])
                    if level == 0:
                        nc.vector.tensor_scalar(CNK[:], PT[:, 0:64], 1.0, 0.0,
                                                Alu.mult, Alu.add,
                                                accum_out=ACC[:, 4 + a:5 + a])
                        continue
                    Par = PT[:].rearrange("p (j c) -> p j c", c=CH)

                    # box path; scratch walk: WB1 pwhh->lt->whc, WB2 plo->rb,
                    # WB3 phi->whr, WS2 pa->ru
                    WB1 = pW.tile([PPART, 2 * W5], f32, name="WB1")
                    WB2 = pW.tile([PPART, 2 * W5], f32, name="WB2")
                    WB3 = pW.tile([PPART, 2 * W5], f32, name="WB3")
                    WS1 = pW.tile([PPART, W5], f32, name="WS1")
                    WS2 = pW.tile([PPART, W5], f32, name="WS2")
                    WS3 = pW.tile([PPART, W5], f32, name="WS3")
                    WS4 = pW.tile([PPART, W5], f32, name="WS4")
                    wb1r = WB1[:].rearrange("p (j c) -> p j c", c=2)
                    wb2r = WB2[:].rearrange("p (j c) -> p j c", c=2)
                    wb3r = WB3[:].rearrange("p (j c) -> p j c", c=2)
                    nc.gpsimd.tensor_scalar(wb1r, Par[:, :, 2:4], 0.5, None,
                                            Alu.mult)                       # pwhh
                    nc.gpsimd.tensor_tensor(wb2r, Par[:, :, 0:2], wb1r,
                                            Alu.subtract)                   # plo
                    nc.gpsimd.tensor_tensor(wb3r, Par[:, :, 0:2], wb1r,
                                            Alu.add)                        # phi
                    nc.vector.tensor_tensor(wb1r, wb2r, tlor, Alu.max)      # lt
                    nc.vector.tensor_tensor(wb2r, wb3r, thir, Alu.min)      # rb
                    nc.gpsimd.tensor_tensor(wb3r, wb2r, wb1r, Alu.subtract)  # whr
                    nc.vector.tensor_scalar(wb1r, wb3r, 0.0, None, Alu.max)  # whc
                    nc.gpsimd.tensor_tensor(WS1[:], wb1r[:, :, 0], wb1r[:, :, 1],
                                            Alu.mult)                       # inter
                    nc.gpsimd.tensor_tensor(WS2[:], Par[:, :, 2], Par[:, :, 3],
                                            Alu.mult)                       # pa
                    nc.gpsimd.tensor_tensor(WS3[:], WS2[:], TA[:], Alu.add)  # s
                    nc.gpsimd.tensor_tensor(WS4[:], WS3[:], WS1[:],
                                            Alu.subtract)                   # union
                    nc.vector.reciprocal_approx_fast(WS2[:], WS4[:])        # ru
                    QC = pQ.tile([PPART, W5], f32, name=f"Q{a}")
                    nc.gpsimd.tensor_tensor(QC[:], WS1[:], WS2[:], Alu.mult)  # q

                    if level < 2:
                        nc.vector.tensor_scalar(CNK[:], QC[:, 0:64], 1.0, 0.0,
                                                Alu.mult, Alu.add,
                                                accum_out=ACC[:, 4 + a:5 + a])
                        qprev = QC
                        Pprev = Par
                        continue

                    if a == 0:
                        qprev = QC
                    else:
                        WM = pQ.tile([PPART, W5], f32, name=f"WM{a}")
                        nc.vector.tensor_tensor(WM[:], qprev[:], QC[:], Alu.is_ge)
                        wib = WM[:].bitcast(i32).unsqueeze(2).broadcast_to(
                            [PPART, W5, CH])
                        nc.vector.copy_predicated(Par, wib, Pprev)
                        if a == 1:
                            QB = pQ.tile([PPART, W5], f32, name="QB")
                            nc.vector.tensor_tensor(QB[:], qprev[:], QC[:], Alu.max)
                            qprev = QB
                    Pprev = Par

                if level >= 2:
                    # ---- bce prep on selected (obj + cls-product) ----
                    LNIN = pPer.tile([PPART, 2 * W5], f32)
                    P2S = pPer.tile([PPART, 4 * W5], f32)
                    p2sr = P2S[:].rearrange("p (j c) -> p j c", c=4)
                    nc.gpsimd.tensor_scalar(LNIN[:, 0:W5], Pprev[:, :, 4], 1.0,
                                            None, Alu.mult)
                    nc.gpsimd.tensor_scalar(p2sr, Pprev[:, :, 0:4], 1.0, None,
                                            Alu.mult)
                    for t in range(d_slices):
                        js = slice(t * LD, (t + 1) * LD)
                        DT = pW.tile([PPART, LD * NCLS], f32, name="DT")
                        dtr = DT[:].rearrange("p (j c) -> p j c", c=NCLS)
                        nc.vector.scalar_tensor_tensor(dtr, Pprev[:, js, 5:CH],
                                                       -1.0, Tr[:, js, 5:CH],
                                                       Alu.add, Alu.add)
                        nc.vector.tensor_reduce(
                            LNIN[:, W5 + t * LD:W5 + (t + 1) * LD], dtr,
                            mybir.AxisListType.X, Alu.mult,
                            apply_absolute_value=True)

                    if level < 3:
                        DMK = pB.tile([PPART, 2 * W5], f32)
                        nc.vector.tensor_scalar(DMK[:], LNIN[:], 1.0, 0.0,
                                                Alu.mult, Alu.add,
                                                accum_out=ACC[:, 5:6])
                        nc.vector.tensor_scalar(DMK[:, 0:W5], MSK[:], 1.0, 0.0,
                                                Alu.mult, Alu.add,
                                                accum_out=ACC[:, 3:4])
                        nc.vector.tensor_scalar(DMK[:], P2S[:, 0:2 * W5], 1.0,
                                                0.0, Alu.mult, Alu.add,
                                                accum_out=ACC[:, 6:7])
                        nc.vector.tensor_scalar(DMK[:], TWH[:], 1.0, 0.0,
                                                Alu.mult, Alu.add,
                                                accum_out=ACC[:, 7:8])
                        nc.vector.tensor_scalar(DMK[:], TCT[:], 1.0, 0.0,
                                                Alu.mult, Alu.add,
                                                accum_out=ACC[:, 2:3])
                    else:
                        # ---- batched post-selection CIoU (525-wide) ----
                        # scratch walk:
                        # B1 pwh2->whr2->chi->sqxy  B2 plo5->cw->lnout
                        # B3 phi5->sqcw  B4 lt2->whc2  B5 rb2->clo->dxy
                        # C1 i->diag->diou->dm2  C2 pa->rdiag->n1->d1->rd2->vn
                        #   ->aden->dm  C3 s->cd->n2->d2->uu->vd0->rvd->raden
                        #   ->cnt  C4 u->qd->z->av->dm3  C5 ru->num->vd->v
                        #   ->cioup  C7 omie
                        B1 = pB.tile([PPART, 2 * W5], f32)
                        B2 = pB.tile([PPART, 2 * W5], f32)
                        B3 = pB.tile([PPART, 2 * W5], f32)
                        B4 = pB.tile([PPART, 2 * W5], f32)
                        B5 = pB.tile([PPART, 2 * W5], f32)
                        C1 = pB.tile([PPART, W5], f32)
                        C2 = pB.tile([PPART, W5], f32)
                        C3 = pB.tile([PPART, W5], f32)
                        C4 = pB.tile([PPART, W5], f32)
                        C5 = pB.tile([PPART, W5], f32)
                        C7 = pB.tile([PPART, W5], f32)
                        b1r = B1[:].rearrange("p (j c) -> p j c", c=2)
                        b2r = B2[:].rearrange("p (j c) -> p j c", c=2)
                        b3r = B3[:].rearrange("p (j c) -> p j c", c=2)
                        b4r = B4[:].rearrange("p (j c) -> p j c", c=2)
                        b5r = B5[:].rearrange("p (j c) -> p j c", c=2)

                        nc.gpsimd.tensor_scalar(b1r, p2sr[:, :, 2:4], 0.5, None,
                                                Alu.mult)                    # pwh2
                        nc.gpsimd.tensor_tensor(b2r, p2sr[:, :, 0:2], b1r,
                                                Alu.subtract)                # plo5
                        nc.gpsimd.tensor_tensor(b3r, p2sr[:, :, 0:2], b1r,
                                                Alu.add)                     # phi5
                        nc.vector.tensor_tensor(b4r, b2r, tlor, Alu.max)     # lt2
                        nc.vector.tensor_tensor(b5r, b3r, thir, Alu.min)     # rb2
                        nc.gpsimd.tensor_tensor(b1r, b5r, b4r, Alu.subtract)  # whr2
                        nc.vector.tensor_scalar(b4r, b1r, 0.0, None, Alu.max)  # whc2
                        nc.gpsimd.tensor_tensor(C1[:], b4r[:, :, 0], b4r[:, :, 1],
                                                Alu.mult)                    # i
                        nc.gpsimd.tensor_tensor(C2[:], p2sr[:, :, 2],
                                                p2sr[:, :, 3], Alu.mult)     # pa
                        nc.gpsimd.tensor_tensor(C3[:], C2[:], TA[:], Alu.add)  # s
                        nc.gpsimd.tensor_tensor(C4[:], C3[:], C1[:],
                                                Alu.subtract)                # u
                        nc.vector.reciprocal_approx_fast(C5[:], C4[:])       # ru
                        nc.gpsimd.tensor_tensor(C2[:], C1[:], C5[:], Alu.mult)
                        # ^ iou -> C2 (pa dead after s)
                        nc.vector.tensor_scalar(C7[:], C2[:], -1.0, 1.0 + EPS,
                                                Alu.mult, Alu.add)           # omie
                        nc.vector.tensor_tensor(b5r, b2r, tlor, Alu.min)     # clo
                        nc.vector.tensor_tensor(b1r, b3r, thir, Alu.max)     # chi
                        nc.gpsimd.tensor_tensor(b2r, b1r, b5r, Alu.subtract)  # cw
                        nc.gpsimd.tensor_tensor(B3[:], B2[:], B2[:], Alu.mult)
                        # ^ sqcw
                        nc.gpsimd.tensor_tensor(C1[:], b3r[:, :, 0], b3r[:, :, 1],
                                                Alu.add)                     # diag
                        nc.vector.reciprocal_approx_fast(C2[:], C1[:])       # rdiag
                        nc.gpsimd.tensor_tensor(b5r, p2sr[:, :, 0:2], tctr,
                                                Alu.subtract)                # dxy
                        nc.gpsimd.tensor_tensor(B1[:], B5[:], B5[:], Alu.mult)
                        # ^ sqxy
                        nc.gpsimd.tensor_tensor(C3[:], b1r[:, :, 0], b1r[:, :, 1],
                                                Alu.add)                     # cd
                        nc.gpsimd.tensor_tensor(C4[:], C3[:], C2[:], Alu.mult)  # qd
                        nc.gpsimd.tensor_tensor(C1[:], C4[:], C7[:], Alu.add)  # diou
                        nc.gpsimd.tensor_tensor(C2[:], twhr[:, :, 0],
                                                p2sr[:, :, 3], Alu.mult)     # n1
                        nc.gpsimd.tensor_tensor(C3[:], p2sr[:, :, 2],
                                                twhr[:, :, 1], Alu.mult)     # n2
                        nc.gpsimd.tensor_tensor(C5[:], C2[:], C3[:],
                                                Alu.subtract)                # num
                        nc.gpsimd.tensor_tensor(C2[:], p2sr[:, :, 3],
                                                twhr[:, :, 1], Alu.mult)     # d1
                        nc.gpsimd.tensor_tensor(C3[:], p2sr[:, :, 2],
                                                twhr[:, :, 0], Alu.mult)     # d2
                        nc.gpsimd.tensor_tensor(C4[:], C2[:], C3[:], Alu.add)
                        # ^ den2
                        nc.vector.reciprocal_approx_fast(C2[:], C4[:])       # rd2
                        nc.gpsimd.tensor_tensor(C3[:], C5[:], C2[:], Alu.mult)  # uu
                        nc.gpsimd.tensor_tensor(C4[:], C3[:], C3[:], Alu.mult)  # z
                        nc.vector.scalar_tensor_tensor(C2[:], C4[:], FB, C4[:],
                                                       Alu.add, Alu.mult)    # vn
                        nc.vector.scalar_tensor_tensor(C3[:], C4[:], FC, C4[:],
                                                       Alu.add, Alu.mult)    # vd0
                        nc.vector.tensor_scalar(C5[:], C3[:], 1.0, FD, Alu.mult,
                                                Alu.add)                     # vd
                        nc.vector.reciprocal_approx_fast(C3[:], C5[:])       # rvd
                        nc.gpsimd.tensor_tensor(C5[:], C2[:], C3[:], Alu.mult)  # v
                        nc.gpsimd.tensor_tensor(C2[:], C5[:], C7[:], Alu.add)
                        # ^ aden
                        nc.vector.reciprocal_approx_fast(C3[:], C2[:])       # raden
                        nc.gpsimd.tensor_tensor(C2[:], C5[:], C5[:], Alu.mult)  # v2
                        nc.gpsimd.tensor_tensor(C4[:], C2[:], C3[:], Alu.mult)  # av
                        nc.gpsimd.tensor_tensor(C5[:], C1[:], C4[:], Alu.add)
                        # ^ cioup
                        nc.vector.scalar_tensor_tensor(C2[:], C5[:], 1.0, MSK[:],
                                                       Alu.mult, Alu.mult,
                                                       accum_out=ACC[:, 0:1])
                        nc.vector.tensor_scalar(C3[:], MSK[:], 1.0, 0.0,
                                                Alu.mult, Alu.add,
                                                accum_out=ACC[:, 3:4])
                        if level == 3:
                            nc.vector.tensor_scalar(B2[:], LNIN[:], 1.0, 0.0,
                                                    Alu.mult, Alu.add,
                                                    accum_out=ACC[:, 5:6])
                        if level >= 4:
                            nc.scalar.activation(B2[:], LNIN[:], Act.Ln)  # lnout
                            nc.vector.scalar_tensor_tensor(
                                C1[:], B2[:, 0:W5], 1.0, MSK[:], Alu.mult,
                                Alu.mult, accum_out=ACC[:, 1:2])
                            nc.vector.scalar_tensor_tensor(
                                C4[:], B2[:, W5:2 * W5], 1.0, MSK[:],
                                Alu.mult, Alu.mult, accum_out=ACC[:, 2:3])

            nc.sync.dma_start(accO, ACC[:])

    nc.compile()
    return nc


def kernel(pred, target):
    pred = np.ascontiguousarray(np.asarray(pred, dtype=np.float32))
    target = np.ascontiguousarray(np.asarray(target, dtype=np.float32))
    assert pred.shape == (B, A, N, CH) and target.shape == (B, N, CH)

    if "nc" not in _CACHE:
        _CACHE["nc"] = _build_bass()
    nc = _CACHE["nc"]

    from concourse import bass_utils

    in_maps = []
    for c in range(NCORES):
        lo, hi = c * BPC, (c + 1) * BPC
        in_maps.append({
            "predL": np.ascontiguousarray(pred[lo:hi]),
            "targL": np.ascontiguousarray(target[lo:hi]),
        })

    res = bass_utils.run_bass_kernel_spmd(nc, in_maps, core_ids=list(range(NCORES)))
    _CACHE["last_results"] = res

    per_batch = []
    for c in range(NCORES):
        acc = res.results[c]["acc_out"].astype(np.float32)   # [128, 8]
        num = acc[:, 0] - acc[:, 1] - 0.1 * acc[:, 2]        # ciou - obj_ln - 0.1*cls_ln
        cnt = acc[:, 3]
        nb = num.reshape(BPC, SEC).sum(axis=1, dtype=np.float32)
        cb = cnt.reshape(BPC, SEC).sum(axis=1, dtype=np.float32)
        per_batch.append(nb / cb)
    loss = np.mean(np.concatenate(per_batch), dtype=np.float32)
    return np.float32(loss)
